# revision 10
# baseline (speedup 1.0000x reference)
"""Quantized int8 3x3 conv (dequant -> conv -> requant) on 8 TRN2 NeuronCores.

Sharding: data-parallel over batch (16 images -> 2 per core), weights/bias
replicated. No cross-core communication.

Per-core Bass kernel strategy:
  - All arithmetic is exact-integer in disguise: (qx - 7) and (qw - 3) are
    8-bit-range integers, exactly representable in bf16; products (<2^16) and
    psum partial sums (<2^24) are exact in fp32. The PE therefore computes the
    integer convolution exactly at full bf16 rate.
  - conv as 6 matmuls per output-row pair (K=128, M=128): input rows are
    stored with row-parity as the partition-dim second half
    (partition = parity*64 + channel), so one [128, N] rhs AP reads two
    consecutive image rows at once.  lhsT packs (parity, channel) x
    (row-of-pair, out-channel) weight taps, zero-padded where the tap is out
    of range (75% non-zero -> 75% PE efficiency; kw taps are free-dim shifts).
  - Both images interleave along the free dim (512-wide matmuls); the
    2 garbage columns at each image edge fall on out-x 254/255 which are
    never stored.
  - Requant is a single DVE tensor_scalar per row pair:
    int32(round(psum * 1e-4 + bias)) — the fp32->int32 writeback rounds to
    nearest-even, matching jnp.round to within exact-.5 scale ties.
"""

import numpy as np

import concourse.bass as bass
import concourse.tile as tile
from concourse import bacc, mybir
from concourse.bass_utils import run_bass_kernel_spmd

N_CORES = 8
IN_ZP = 7
W_ZP = 3
SCALE = 1e-4  # IN_SCALE * W_SCALE; OUT_SCALE=1, OUT_ZP=0, B_SCALE=1, B_ZP=0


def build_nc(H=256, W=256, n_img=2, n_cores=N_CORES, iters=1):
    C = 64   # input channels
    CO = 64  # output channels
    OH, OW = H - 2, W - 2
    assert H % 8 == 0
    n_blocks = H // 2          # row-pair blocks (parity-packed)
    n_pairs = OH // 2          # output row pairs
    n_groups = n_blocks // 4   # a group tile holds 4 blocks
    n_units = (n_pairs + 3) // 4
    BLK = n_img * W            # free-dim columns per block (images interleaved)
    GCOL = 4 * BLK

    nc = bacc.Bacc("TRN2", target_bir_lowering=False, debug=False,
                   num_devices=n_cores)
    x = nc.declare_dram_parameter("x", [n_img, C, H, W], mybir.dt.int32,
                                  isOutput=False)
    w = nc.declare_dram_parameter("w", [CO, C, 3, 3], mybir.dt.int32,
                                  isOutput=False)
    b = nc.declare_dram_parameter("b", [CO], mybir.dt.int32, isOutput=False)
    y = nc.declare_dram_parameter("y", [n_img, CO, OH, OW], mybir.dt.int32,
                                  isOutput=True)

    with tile.TileContext(nc) as tc:
        with (
            tc.tile_pool(name="const", bufs=1) as constp,
            tc.tile_pool(name="stage", bufs=6) as stagep,
            tc.tile_pool(name="x2", bufs=6) as x2p,
            tc.tile_pool(name="outp", bufs=6) as outp,
            tc.tile_pool(name="psum", bufs=8, space="PSUM") as psp,
        ):
            # ---- weight prep: SBUF[c + 64*par, tap*64 + o] = w[o, c, kh, kw]
            w_i32 = constp.tile([128, 9 * CO], mybir.dt.int32, tag="w_i32")
            wsrc = w.rearrange("o c kh kw -> c (kh kw) o")  # [64, 9, 64]
            wv = w_i32.rearrange("p (t o) -> p t o", t=9)
            nc.sync.dma_start(wv[0:64], wsrc)
            nc.sync.dma_start(wv[64:128], wsrc)
            w16 = constp.tile([128, 9 * CO], mybir.dt.bfloat16, tag="w16")
            nc.vector.tensor_scalar(w16[:], w_i32[:], W_ZP, None,
                                    mybir.AluOpType.subtract)

            # lhsT[j2*3+kw][par*64 + c, r*64 + o] = w16[c, kh=2*j2+par-r, kw, o]
            lhs = []
            for j2 in range(2):
                for kw in range(3):
                    t = constp.tile([128, 128], mybir.dt.bfloat16,
                                    tag=f"lhs{j2}{kw}")
                    nc.vector.memset(t[:], 0.0)
                    for par in range(2):
                        for r in range(2):
                            kh = 2 * j2 + par - r
                            if 0 <= kh <= 2:
                                tap = kh * 3 + kw
                                nc.vector.tensor_copy(
                                    t[par * 64:(par + 1) * 64,
                                      r * 64:(r + 1) * 64],
                                    w16[par * 64:(par + 1) * 64,
                                        tap * 64:(tap + 1) * 64])
                    lhs.append(t)

            # ---- bias: [128,1] f32, partition = r*64 + o (b repeated twice)
            b_i32 = constp.tile([128, 1], mybir.dt.int32, tag="b_i32")
            bsrc = b.rearrange("(o u) -> o u", u=1)
            nc.sync.dma_start(b_i32[0:64], bsrc)
            nc.sync.dma_start(b_i32[64:128], bsrc)
            bias_f = constp.tile([128, 1], mybir.dt.float32, tag="bias_f")
            nc.vector.tensor_copy(bias_f[:], b_i32[:])

            # ---- input view: [parity][c, pair_row, x]; the (img, parity)
            # DMAs land on complementary partition halves and run
            # concurrently, so together they use all 16 SBUF ports
            x_par = [x[i].rearrange("c (rp two) w -> two c rp w", two=2)
                     for i in range(n_img)]

            x2_tiles = {}

            def load_group(g):
                st = stagep.tile([128, GCOL], mybir.dt.int32, tag="stage")
                stv = st.rearrange("p (blk i w) -> p blk i w", blk=4, i=n_img)
                for img in range(n_img):
                    for par in range(2):
                        src = x_par[img][par][:, 4 * g:4 * g + 4, :]
                        nc.sync.dma_start(
                            stv[par * 64:(par + 1) * 64, :, img, :], src)
                xt = x2p.tile([128, GCOL], mybir.dt.bfloat16, tag="x2")
                # dequant convert on GpSimd: 1-input elemwise runs at line
                # rate there and frees DVE for requant
                nc.gpsimd.tensor_scalar(xt[:], st[:], IN_ZP, None,
                                        mybir.AluOpType.subtract)
                x2_tiles[g] = xt

            def compute_unit(u):
                pairs = list(range(4 * u, min(4 * u + 4, n_pairs)))
                nq = len(pairs)
                ps = [psp.tile([128, BLK], mybir.dt.float32, tag="ps",
                               name=f"ps_{u}_{i}")
                      for i in range(nq)]
                for j2 in range(2):
                    for kw in range(3):
                        lt = lhs[j2 * 3 + kw]
                        for q, pair in enumerate(pairs):
                            g2, lb = divmod(pair + j2, 4)
                            rhs = x2_tiles[g2][:, lb * BLK + kw:
                                               lb * BLK + BLK]
                            nc.tensor.matmul(
                                ps[q][:, 0:BLK - kw], lt[:], rhs,
                                start=(j2 == 0 and kw == 0),
                                stop=(j2 == 1 and kw == 2))
                ot = outp.tile([128, 4 * BLK], mybir.dt.int32, tag="out")
                for q in range(nq):
                    nc.vector.tensor_scalar(
                        ot[:, q * BLK:(q + 1) * BLK], ps[q][:],
                        SCALE, bias_f[:],
                        mybir.AluOpType.mult, mybir.AluOpType.add)
                otv = ot.rearrange("p (qq i w) -> p qq i w", qq=4, i=n_img)
                for img in range(n_img):
                    for r in range(2):
                        src = otv[r * 64:(r + 1) * 64, 0:nq, img, 0:OW]
                        r0 = 8 * u + r
                        dst = y[img][:, r0: r0 + 2 * nq - 1: 2, :]
                        nc.scalar.dma_start(dst, src)

            def main_body():
                x2_tiles.clear()
                load_group(0)
                if n_groups > 1:
                    load_group(1)
                for u in range(n_units):
                    compute_unit(u)
                    if u + 2 < n_groups:
                        load_group(u + 2)

            if iters == 1:
                main_body()
            else:
                # benchmarking variant: repeat the whole streaming body on
                # device so per-iteration HW time can be extracted from the
                # wall-clock delta between two NEFFs
                with tc.For_i(0, iters, 1):
                    main_body()

    nc.compile()
    return nc


_NC_CACHE = {}


def get_nc(H=256, W=256, n_img=2):
    key = (H, W, n_img)
    if key not in _NC_CACHE:
        _NC_CACHE[key] = build_nc(H, W, n_img)
    return _NC_CACHE[key]


def run_sharded(nc, input, weight, bias, n_img, **kwargs):
    input = np.ascontiguousarray(input, dtype=np.int32)
    weight = np.ascontiguousarray(weight, dtype=np.int32)
    bias = np.ascontiguousarray(bias, dtype=np.int32)
    in_maps = [
        {"x": input[i * n_img:(i + 1) * n_img], "w": weight, "b": bias}
        for i in range(N_CORES)
    ]
    res = run_bass_kernel_spmd(nc, in_maps, list(range(N_CORES)), **kwargs)
    out = np.concatenate([r["y"] for r in res.results], axis=0)
    return out.astype(np.int32, copy=False), res


def kernel(input, weight, bias):
    n_img = input.shape[0] // N_CORES
    nc = get_nc(input.shape[2], input.shape[3], n_img)
    out, _ = run_sharded(nc, input, weight, bias, n_img)
    return out


# revision 12
# speedup vs baseline: 2.9972x; 2.9972x over previous
"""Quantized int8 3x3 conv (dequant -> conv -> requant) on 8 TRN2 NeuronCores.

Sharding: data-parallel over batch (16 images -> 2 per core), weights/bias
replicated. No cross-core communication.

Per-core Bass kernel strategy:
  - All arithmetic is exact-integer in disguise: (qx - 7) and (qw - 3) are
    8-bit-range integers, exactly representable in bf16; products (<2^16) and
    psum partial sums (<2^24) are exact in fp32. The PE therefore computes the
    integer convolution exactly at full bf16 rate.
  - conv as 6 matmuls per output-row pair (K=128, M=128): input rows are
    stored with row-parity as the partition-dim second half
    (partition = parity*64 + channel), so one [128, N] rhs AP reads two
    consecutive image rows at once.  lhsT packs (parity, channel) x
    (row-of-pair, out-channel) weight taps, zero-padded where the tap is out
    of range (75% non-zero -> 75% PE efficiency; kw taps are free-dim shifts).
  - Both images interleave along the free dim (512-wide matmuls); the
    2 garbage columns at each image edge fall on out-x 254/255 which are
    never stored.
  - Requant is a single DVE tensor_scalar per row pair:
    int32(round(psum * 1e-4 + bias)) — the fp32->int32 writeback rounds to
    nearest-even, matching jnp.round to within exact-.5 scale ties.
"""

import numpy as np

import concourse.bass as bass
import concourse.tile as tile
from concourse import bacc, mybir
from concourse.bass_utils import run_bass_kernel_spmd

N_CORES = 8
IN_ZP = 7
W_ZP = 3
SCALE = 1e-4  # IN_SCALE * W_SCALE; OUT_SCALE=1, OUT_ZP=0, B_SCALE=1, B_ZP=0


def build_nc(H=256, W=256, n_img=2, n_cores=N_CORES, iters=1,
             convert_engine="dve"):
    C = 64   # input channels
    CO = 64  # output channels
    OH, OW = H - 2, W - 2
    assert H % 8 == 0
    n_blocks = H // 2          # row-pair blocks (parity-packed)
    n_pairs = OH // 2          # output row pairs
    n_groups = n_blocks // 4   # a group tile holds 4 blocks
    n_units = (n_pairs + 3) // 4
    BLK = n_img * W            # free-dim columns per block (images interleaved)
    GCOL = 4 * BLK

    nc = bacc.Bacc("TRN2", target_bir_lowering=False, debug=False,
                   num_devices=n_cores)
    x = nc.declare_dram_parameter("x", [n_img, C, H, W], mybir.dt.int32,
                                  isOutput=False)
    w = nc.declare_dram_parameter("w", [CO, C, 3, 3], mybir.dt.int32,
                                  isOutput=False)
    b = nc.declare_dram_parameter("b", [CO], mybir.dt.int32, isOutput=False)
    # output values are bounded by |0.0001*576*135*131 + 1000| < 2^15, so
    # int16 storage is lossless and halves output DMA traffic; the host
    # widens to int32 after the gather
    y = nc.declare_dram_parameter("y", [n_img, CO, OH, OW], mybir.dt.int16,
                                  isOutput=True)

    with tile.TileContext(nc) as tc:
        with (
            tc.tile_pool(name="const", bufs=1) as constp,
            tc.tile_pool(name="stage", bufs=6) as stagep,
            tc.tile_pool(name="x2", bufs=6) as x2p,
            tc.tile_pool(name="outp", bufs=6) as outp,
            tc.tile_pool(name="psum", bufs=8, space="PSUM") as psp,
        ):
            # ---- weight prep: SBUF[c + 64*par, tap*64 + o] = w[o, c, kh, kw]
            w_i32 = constp.tile([128, 9 * CO], mybir.dt.int32, tag="w_i32")
            wsrc = w.rearrange("o c kh kw -> c (kh kw) o")  # [64, 9, 64]
            wv = w_i32.rearrange("p (t o) -> p t o", t=9)
            nc.sync.dma_start(wv[0:64], wsrc)
            nc.sync.dma_start(wv[64:128], wsrc)
            w16 = constp.tile([128, 9 * CO], mybir.dt.bfloat16, tag="w16")
            nc.vector.tensor_scalar(w16[:], w_i32[:], W_ZP, None,
                                    mybir.AluOpType.subtract)

            # lhsT[j2*3+kw][par*64 + c, r*64 + o] = w16[c, kh=2*j2+par-r, kw, o]
            lhs = []
            for j2 in range(2):
                for kw in range(3):
                    t = constp.tile([128, 128], mybir.dt.bfloat16,
                                    tag=f"lhs{j2}{kw}")
                    nc.vector.memset(t[:], 0.0)
                    for par in range(2):
                        for r in range(2):
                            kh = 2 * j2 + par - r
                            if 0 <= kh <= 2:
                                tap = kh * 3 + kw
                                nc.vector.tensor_copy(
                                    t[par * 64:(par + 1) * 64,
                                      r * 64:(r + 1) * 64],
                                    w16[par * 64:(par + 1) * 64,
                                        tap * 64:(tap + 1) * 64])
                    lhs.append(t)

            # ---- bias: [128,1] f32, partition = r*64 + o (b repeated twice)
            b_i32 = constp.tile([128, 1], mybir.dt.int32, tag="b_i32")
            bsrc = b.rearrange("(o u) -> o u", u=1)
            nc.sync.dma_start(b_i32[0:64], bsrc)
            nc.sync.dma_start(b_i32[64:128], bsrc)
            bias_f = constp.tile([128, 1], mybir.dt.float32, tag="bias_f")
            nc.vector.tensor_copy(bias_f[:], b_i32[:])

            # ---- input view: [parity][c, pair_row, x]; the (img, parity)
            # DMAs land on complementary partition halves and run
            # concurrently, so together they use all 16 SBUF ports
            x_par = [x[i].rearrange("c (rp two) w -> two c rp w", two=2)
                     for i in range(n_img)]

            x2_tiles = {}

            def load_group(g):
                st = stagep.tile([128, GCOL], mybir.dt.int32, tag="stage")
                stv = st.rearrange("p (blk i w) -> p blk i w", blk=4, i=n_img)
                for img in range(n_img):
                    for par in range(2):
                        src = x_par[img][par][:, 4 * g:4 * g + 4, :]
                        nc.sync.dma_start(
                            stv[par * 64:(par + 1) * 64, :, img, :], src)
                xt = x2p.tile([128, GCOL], mybir.dt.bfloat16, tag="x2")
                if convert_engine == "act":
                    # Copy path allows a float bias; values <= 135 are exact
                    # even at reduced internal precision
                    nc.scalar.activation(xt[:], st[:],
                                         mybir.ActivationFunctionType.Copy,
                                         bias=-float(IN_ZP), scale=1.0)
                else:
                    nc.vector.tensor_scalar(xt[:], st[:], IN_ZP, None,
                                            mybir.AluOpType.subtract)
                x2_tiles[g] = xt

            def compute_unit(u):
                pairs = list(range(4 * u, min(4 * u + 4, n_pairs)))
                nq = len(pairs)
                ps = [psp.tile([128, BLK], mybir.dt.float32, tag="ps",
                               name=f"ps_{u}_{i}")
                      for i in range(nq)]
                for j2 in range(2):
                    for kw in range(3):
                        lt = lhs[j2 * 3 + kw]
                        for q, pair in enumerate(pairs):
                            g2, lb = divmod(pair + j2, 4)
                            rhs = x2_tiles[g2][:, lb * BLK + kw:
                                               lb * BLK + BLK]
                            nc.tensor.matmul(
                                ps[q][:, 0:BLK - kw], lt[:], rhs,
                                start=(j2 == 0 and kw == 0),
                                stop=(j2 == 1 and kw == 2))
                ot = outp.tile([128, 4 * BLK], mybir.dt.int16, tag="out")
                for q in range(nq):
                    nc.vector.tensor_scalar(
                        ot[:, q * BLK:(q + 1) * BLK], ps[q][:],
                        SCALE, bias_f[:],
                        mybir.AluOpType.mult, mybir.AluOpType.add)
                otv = ot.rearrange("p (qq i w) -> p qq i w", qq=4, i=n_img)
                for img in range(n_img):
                    for r in range(2):
                        src = otv[r * 64:(r + 1) * 64, 0:nq, img, 0:OW]
                        r0 = 8 * u + r
                        dst = y[img][:, r0: r0 + 2 * nq - 1: 2, :]
                        nc.scalar.dma_start(dst, src)

            def main_body():
                x2_tiles.clear()
                load_group(0)
                if n_groups > 1:
                    load_group(1)
                for u in range(n_units):
                    compute_unit(u)
                    if u + 2 < n_groups:
                        load_group(u + 2)

            if iters == 1:
                main_body()
            else:
                # benchmarking variant: repeat the whole streaming body on
                # device so per-iteration HW time can be extracted from the
                # wall-clock delta between two NEFFs
                with tc.For_i(0, iters, 1):
                    main_body()

    nc.compile()
    return nc


_NC_CACHE = {}


def get_nc(H=256, W=256, n_img=2):
    key = (H, W, n_img)
    if key not in _NC_CACHE:
        _NC_CACHE[key] = build_nc(H, W, n_img)
    return _NC_CACHE[key]


def run_sharded(nc, input, weight, bias, n_img, **kwargs):
    input = np.ascontiguousarray(input, dtype=np.int32)
    weight = np.ascontiguousarray(weight, dtype=np.int32)
    bias = np.ascontiguousarray(bias, dtype=np.int32)
    in_maps = [
        {"x": input[i * n_img:(i + 1) * n_img], "w": weight, "b": bias}
        for i in range(N_CORES)
    ]
    res = run_bass_kernel_spmd(nc, in_maps, list(range(N_CORES)), **kwargs)
    out = np.concatenate([r["y"] for r in res.results], axis=0)
    return out.astype(np.int32), res


def kernel(input, weight, bias):
    n_img = input.shape[0] // N_CORES
    nc = get_nc(input.shape[2], input.shape[3], n_img)
    out, _ = run_sharded(nc, input, weight, bias, n_img)
    return out


# revision 15
# speedup vs baseline: 6.0190x; 2.0082x over previous
"""Quantized int8 3x3 conv (dequant -> conv -> requant) on 8 TRN2 NeuronCores.

Sharding: data-parallel over batch (16 images -> 2 per core), weights/bias
replicated. No cross-core communication.

Per-core Bass kernel strategy:
  - All arithmetic is exact-integer in disguise: (qx - 7) and (qw - 3) are
    8-bit-range integers, exactly representable in bf16; products (<2^16) and
    psum partial sums (<2^24) are exact in fp32. The PE therefore computes the
    integer convolution exactly at full bf16 rate.
  - conv as 6 matmuls per output-row pair (K=128, M=128): input rows are
    stored with row-parity as the partition-dim second half
    (partition = parity*64 + channel), so one [128, N] rhs AP reads two
    consecutive image rows at once.  lhsT packs (parity, channel) x
    (row-of-pair, out-channel) weight taps, zero-padded where the tap is out
    of range (75% non-zero -> 75% PE efficiency; kw taps are free-dim shifts).
  - Both images interleave along the free dim (512-wide matmuls); the
    2 garbage columns at each image edge fall on out-x 254/255 which are
    never stored.
  - Requant is a single DVE tensor_scalar per row pair:
    int32(round(psum * 1e-4 + bias)) — the fp32->int32 writeback rounds to
    nearest-even, matching jnp.round to within exact-.5 scale ties.
"""

import os

import numpy as np

import concourse.bass as bass
import concourse.tile as tile
from concourse import bacc, mybir
from concourse.bass_utils import run_bass_kernel_spmd

N_CORES = 8
IN_ZP = 7
W_ZP = 3
SCALE = 1e-4  # IN_SCALE * W_SCALE; OUT_SCALE=1, OUT_ZP=0, B_SCALE=1, B_ZP=0


def build_nc(H=256, W=256, n_img=2, n_cores=N_CORES, iters=1,
             convert_engine=os.environ.get("CONV_ENGINE", "dve"),
             stages=os.environ.get("STAGES", "all")):
    C = 64   # input channels
    CO = 64  # output channels
    OH, OW = H - 2, W - 2
    assert H % 8 == 0
    n_blocks = H // 2          # row-pair blocks (parity-packed)
    n_pairs = OH // 2          # output row pairs
    n_groups = n_blocks // 4   # a group tile holds 4 blocks
    n_units = (n_pairs + 3) // 4
    BLK = n_img * W            # free-dim columns per block (images interleaved)
    GCOL = 4 * BLK

    nc = bacc.Bacc("TRN2", target_bir_lowering=False, debug=False,
                   num_devices=n_cores)
    x = nc.declare_dram_parameter("x", [n_img, C, H, W], mybir.dt.int32,
                                  isOutput=False)
    w = nc.declare_dram_parameter("w", [CO, C, 3, 3], mybir.dt.int32,
                                  isOutput=False)
    b = nc.declare_dram_parameter("b", [CO], mybir.dt.int32, isOutput=False)
    # output values are bounded by |0.0001*576*135*131 + 1000| < 2^15, so
    # int16 storage is lossless and halves output DMA traffic; the host
    # widens to int32 after the gather
    y = nc.declare_dram_parameter("y", [n_img, CO, OH, OW], mybir.dt.int16,
                                  isOutput=True)

    with tile.TileContext(nc) as tc:
        with (
            tc.tile_pool(name="const", bufs=1) as constp,
            tc.tile_pool(name="stage", bufs=6) as stagep,
            tc.tile_pool(name="x2", bufs=6) as x2p,
            tc.tile_pool(name="outp", bufs=6) as outp,
            tc.tile_pool(name="psum", bufs=8, space="PSUM") as psp,
        ):
            # ---- weight prep: SBUF[c + 64*par, tap*64 + o] = w[o, c, kh, kw]
            w_i32 = constp.tile([128, 9 * CO], mybir.dt.int32, tag="w_i32")
            wsrc = w.rearrange("o c kh kw -> c (kh kw) o")  # [64, 9, 64]
            wv = w_i32.rearrange("p (t o) -> p t o", t=9)
            nc.sync.dma_start(wv[0:64], wsrc)
            nc.sync.dma_start(wv[64:128], wsrc)
            w16 = constp.tile([128, 9 * CO], mybir.dt.bfloat16, tag="w16")
            nc.vector.tensor_scalar(w16[:], w_i32[:], W_ZP, None,
                                    mybir.AluOpType.subtract)

            # lhsT[j2*3+kw][par*64 + c, r*64 + o] = w16[c, kh=2*j2+par-r, kw, o]
            lhs = []
            for j2 in range(2):
                for kw in range(3):
                    t = constp.tile([128, 128], mybir.dt.bfloat16,
                                    tag=f"lhs{j2}{kw}")
                    nc.vector.memset(t[:], 0.0)
                    for par in range(2):
                        for r in range(2):
                            kh = 2 * j2 + par - r
                            if 0 <= kh <= 2:
                                tap = kh * 3 + kw
                                nc.vector.tensor_copy(
                                    t[par * 64:(par + 1) * 64,
                                      r * 64:(r + 1) * 64],
                                    w16[par * 64:(par + 1) * 64,
                                        tap * 64:(tap + 1) * 64])
                    lhs.append(t)

            # ---- bias: [128,1] f32, partition = r*64 + o (b repeated twice)
            b_i32 = constp.tile([128, 1], mybir.dt.int32, tag="b_i32")
            bsrc = b.rearrange("(o u) -> o u", u=1)
            nc.sync.dma_start(b_i32[0:64], bsrc)
            nc.sync.dma_start(b_i32[64:128], bsrc)
            bias_f = constp.tile([128, 1], mybir.dt.float32, tag="bias_f")
            nc.vector.tensor_copy(bias_f[:], b_i32[:])

            # ---- input view: [parity][c, pair_row, x]; the (img, parity)
            # DMAs land on complementary partition halves and run
            # concurrently, so together they use all 16 SBUF ports
            x_par = [x[i].rearrange("c (rp two) w -> two c rp w", two=2)
                     for i in range(n_img)]

            x2_tiles = {}

            def load_group(g):
                st = stagep.tile([128, GCOL], mybir.dt.int32, tag="stage")
                stv = st.rearrange("p (blk i w) -> p blk i w", blk=4, i=n_img)
                for img in range(n_img):
                    for par in range(2):
                        src = x_par[img][par][:, 4 * g:4 * g + 4, :]
                        nc.sync.dma_start(
                            stv[par * 64:(par + 1) * 64, :, img, :], src)
                xt = x2p.tile([128, GCOL], mybir.dt.bfloat16, tag="x2")
                if convert_engine == "act":
                    # Copy path allows a float bias; values <= 135 are exact
                    # even at reduced internal precision
                    nc.scalar.activation(xt[:], st[:],
                                         mybir.ActivationFunctionType.Copy,
                                         bias=-float(IN_ZP), scale=1.0)
                else:
                    nc.vector.tensor_scalar(xt[:], st[:], IN_ZP, None,
                                            mybir.AluOpType.subtract)
                x2_tiles[g] = xt

            def compute_unit(u):
                if stages == "input":
                    # attribution variant: keep one tiny consumer so DCE
                    # can't eliminate the loads/converts
                    if u == n_units - 1:
                        ot = outp.tile([128, 4 * BLK], mybir.dt.int16,
                                       tag="out")
                        nc.vector.tensor_scalar(
                            ot[:, 0:GCOL], x2_tiles[4 * u // 4][:],
                            1, None, mybir.AluOpType.mult)
                        nc.scalar.dma_start(
                            y[0][:, 0:1, 0:OW].rearrange("o h w -> o (h w)"),
                            ot[0:64, 0:OW])
                    return
                pairs = list(range(4 * u, min(4 * u + 4, n_pairs)))
                nq = len(pairs)
                ps = [psp.tile([128, BLK], mybir.dt.float32, tag="ps",
                               name=f"ps_{u}_{i}")
                      for i in range(nq)]
                for j2 in range(2):
                    for kw in range(3):
                        lt = lhs[j2 * 3 + kw]
                        for q, pair in enumerate(pairs):
                            g2, lb = divmod(pair + j2, 4)
                            rhs = x2_tiles[g2][:, lb * BLK + kw:
                                               lb * BLK + BLK]
                            nc.tensor.matmul(
                                ps[q][:, 0:BLK - kw], lt[:], rhs,
                                start=(j2 == 0 and kw == 0),
                                stop=(j2 == 1 and kw == 2))
                if stages == "noout":
                    if u != n_units - 1:
                        return
                ot = outp.tile([128, 4 * BLK], mybir.dt.int16, tag="out")
                for q in range(nq):
                    nc.vector.tensor_scalar(
                        ot[:, q * BLK:(q + 1) * BLK], ps[q][:],
                        SCALE, bias_f[:],
                        mybir.AluOpType.mult, mybir.AluOpType.add)
                otv = ot.rearrange("p (qq i w) -> p qq i w", qq=4, i=n_img)
                for img in range(n_img):
                    for r in range(2):
                        src = otv[r * 64:(r + 1) * 64, 0:nq, img, 0:OW]
                        r0 = 8 * u + r
                        dst = y[img][:, r0: r0 + 2 * nq - 1: 2, :]
                        nc.scalar.dma_start(dst, src)

            def main_body():
                x2_tiles.clear()
                load_group(0)
                if n_groups > 1:
                    load_group(1)
                for u in range(n_units):
                    compute_unit(u)
                    if u + 2 < n_groups:
                        load_group(u + 2)

            if iters == 1:
                main_body()
            else:
                # benchmarking variant: repeat the whole streaming body on
                # device so per-iteration HW time can be extracted from the
                # wall-clock delta between two NEFFs
                with tc.For_i(0, iters, 1):
                    main_body()

    nc.compile()
    return nc


_NC_CACHE = {}


def get_nc(H=256, W=256, n_img=2):
    key = (H, W, n_img)
    if key not in _NC_CACHE:
        _NC_CACHE[key] = build_nc(H, W, n_img)
    return _NC_CACHE[key]


def run_sharded(nc, input, weight, bias, n_img, **kwargs):
    input = np.ascontiguousarray(input, dtype=np.int32)
    weight = np.ascontiguousarray(weight, dtype=np.int32)
    bias = np.ascontiguousarray(bias, dtype=np.int32)
    in_maps = [
        {"x": input[i * n_img:(i + 1) * n_img], "w": weight, "b": bias}
        for i in range(N_CORES)
    ]
    res = run_bass_kernel_spmd(nc, in_maps, list(range(N_CORES)), **kwargs)
    out = np.concatenate([r["y"] for r in res.results], axis=0)
    return out.astype(np.int32), res


def kernel(input, weight, bias):
    n_img = input.shape[0] // N_CORES
    nc = get_nc(input.shape[2], input.shape[3], n_img)
    out, _ = run_sharded(nc, input, weight, bias, n_img)
    return out
